# revision 22
# baseline (speedup 1.0000x reference)
"""Cross-attention kernel for Trainium2 (Bass/Tile), 8-core data-parallel, v9i.

Reference computation (per batch element b):
    q = x @ Wq.T ; k = ctx @ Wk.T ; v = ctx @ Wv.T
    out = softmax((q @ k.T) * D**-0.5) @ v

Shapes: x [8, 2048, 1024], context [8, 2048, 1024], Wq/Wk/Wv [1024, 1024].

v9 changes over the 419us v8:
 - The 8-core-parallel input stream runs at 240-300GB/s per core
   run-to-run (chip-level HBM contention); six tb0-3 ne1 quarter-chunks
   are held back as reserve work and dripped through the ctx4-7/wq/wk
   arrival waits so slow runs don't open >=3.4us idle windows (which
   would re-throttle the HAM clock gate mid-kernel).
 - v-projection chunks for t-blocks 0-3 are 256 cols wide (one wv pack
   each) so every arriving pack unlocks PE work immediately; the early
   schedule is re-laid-out against measured pack arrival times and keeps
   the PE busy enough that the HAM clock gate never re-throttles.
 - A 260-matmul warmup burst (~7us continuous, a full free-running HAM
   SHORT window) flips the clock gate to 8/8 by ~14us; small fillers sit
   at the three early cast-wait sites so no >=3.4us idle window ever
   re-throttles it (v8 ran its whole staging ramp at 1.2GHz until 23us).
 - Mid-phase ctx transposes are scheduled the moment each pack lands:
   the stage(2)/castp(6) rings gate later DMA issues on those transpose
   reads, so deferring them throttled the wq/wk stream by ~15us (v9c).
 - Softmax denominators: DVE tree-reduces atT 16->8 then in-place 8->4,
   so the PE ones-matmul chain is 4 deep instead of 8 (v8 had 16 v7-style
   pair inputs); no extra SBUF (only ~1.5KB headroom exists).
 - attnv output ships per-jj ([128,1024] fp32, 4KB lines) alternating
   between the sync and gpsimd DMA queues; the final s-block ships per-ne
   256KB chunks so the exit tail is mul+0.9us instead of a 1MB drain.
 - x^T staging for s-half 1 (packs 4-7) moves into the dots(h0) phase,
   removing the x-pack DMA edge during the W' phase.
"""

from contextlib import ExitStack

import numpy as np

B = 8
S = 2048  # query length
T = 2048  # key/value length
D = 1024  # model dim
P = 128
SCALE = float(D) ** -0.5

N_TB = T // P  # 16 key/value t-blocks
N_DT = D // P  # 8 contraction chunks
NPROJ = D // 512  # 2 x 512-wide chunks for [., 1024] outputs
SH = S // 2  # s processed in two halves of 1024

RP2 = 2  # DRAM rows packed per SBUF partition (8KB descriptors)
NPK_WV = D // (P * RP2)  # 4 wv packs (e-blocks of 256)
NPK_CTX = T // (P * RP2)  # 8 ctx packs (t-blocks of 256)
NPK_W = D // (P * RP2)  # 4 wq/wk packs (e-blocks of 256)
NPK_X = S // (P * RP2)  # 8 x packs (s-blocks of 256)

# NOTE: sharding W' across cores + AllGather was measured and rejected: a
# 2MB 8-rank AllGather costs ~95us end-to-end (ring, ~10us/step fixed), far
# above the 24us of PE time it would save.


def _emit_body(tc, x, ctxt, wq, wk, wv, out):
    import concourse.mybir as mybir
    from concourse.masks import make_identity

    fp32 = mybir.dt.float32
    bf16 = mybir.dt.bfloat16
    nc = tc.nc

    with ExitStack() as ctx:
        const = ctx.enter_context(tc.tile_pool(name="const", bufs=1))
        stage = ctx.enter_context(tc.tile_pool(name="stage", bufs=2))
        castp = ctx.enter_context(tc.tile_pool(name="castp", bufs=6))
        wnp = ctx.enter_context(tc.tile_pool(name="wnp", bufs=16))
        big8 = ctx.enter_context(tc.tile_pool(name="big8", bufs=4))
        ctp = ctx.enter_context(tc.tile_pool(name="ctp", bufs=16))
        wpp = ctx.enter_context(tc.tile_pool(name="wpp", bufs=8))
        vp = ctx.enter_context(tc.tile_pool(name="vp", bufs=16))
        ytp = ctx.enter_context(tc.tile_pool(name="ytp", bufs=8))
        smp = ctx.enter_context(tc.tile_pool(name="smp", bufs=2))
        sump = ctx.enter_context(tc.tile_pool(name="sump", bufs=8))

        ones_b = const.tile([P, 1], bf16, name="ones_b")
        nc.vector.memset(ones_b, 1.0)
        ident_1 = const.tile([1, 1], bf16, name="ident_1")
        nc.vector.memset(ident_1, 1.0)
        ident_b = const.tile([P, P], bf16, name="ident_b")
        make_identity(nc, ident_b)
        warm_b = const.tile([P, P], bf16, name="warm_b")
        nc.vector.memset(warm_b, 0.125)

        # wvg2[g] [128, 4, 1024]: Wv^T d-chunks 4g+cc on planes, e natural.
        # ctxT[tb] [128, 8, 128]: ctx^T t-block tb = 2*pk + j, d-chunk planes,
        #   with t = 256*pk + 2*f + j at free position f.
        # xtb[sb] [128, 4, 8, 128]: x^T for 512-col s-block sb, plane (jj, c);
        #   s = 512*sb + 256*(jj//2) + 2*f + (jj%2).
        wvg2 = [
            big8.tile([P, 4, D], bf16, name=f"wvg{g}", tag="big") for g in range(2)
        ]
        ctxT = [
            ctp.tile([P, N_DT, P], bf16, name=f"ctxT{tb}", tag="ct")
            for tb in range(N_TB)
        ]
        v = [vp.tile([P, D], bf16, name=f"v{tb}", tag="v") for tb in range(N_TB)]

        with tc.tile_pool(name="psum", bufs=1, space="PSUM") as ps_pool:
            # dummy matmuls keep the HAM clock gate busy through the DMA head
            # (transpose-mode does not count as PE-busy for the HAM).
            warm_ps = ps_pool.tile([P, 512], fp32, name="warm", tag="mm", bufs=5)

            def filler(n):
                # full 128x128 array activity per dummy (~56-107ns each); the
                # HAM does not count near-empty 1x1 matmuls as PE-busy
                for _ in range(n):
                    nc.tensor.matmul(
                        warm_ps[:, 0:P], warm_b, warm_b, start=True, stop=True
                    )

            # tiny transfers absorb the DMA-engine ramp before the first pack
            scr = const.tile([1, 64], fp32, name="scr")
            nc.sync.dma_start(out=scr, in_=wv[0:1, 0:64])
            scr2 = const.tile([1, 64], fp32, name="scr2")
            nc.gpsimd.dma_start(out=scr2, in_=ctxt[0:1, 0:64])

            # ~4us of continuous full-array dummies guarantees a complete
            # free-running HAM SHORT window -> PE warm from ~11us.
            filler(40)

            # ------ input staging: DMA (row-packed) -> cast -> transpose ----
            drain_eng = [0]

            # all input DMAs stay on the single sync queue: a dual-queue
            # experiment (v9e) showed pathological interference (a 1MB pack
            # taking 13.7us next to a parallel 5.4us one) and later arrivals
            # overall.
            def in_dma(out, in_):
                nc.sync.dma_start(out=out, in_=in_)

            def load_pack(dram_t, npk, rp, pk, nm):
                """DMA rows [pk*128*rp, (pk+1)*128*rp) as [128, rp, 1024] fp32
                (partition p holds rows 128*rp*pk + rp*p + {0..rp-1}), cast
                each row-plane to bf16. Returns the rp bf16 [128, D] planes."""
                st = stage.tile([P, rp, D], fp32, name=f"st_{nm}", tag="stage")
                src = dram_t.rearrange("(k p j) d -> k p j d", k=npk, p=P, j=rp)
                in_dma(st, src[pk])
                planes = []
                for j in range(rp):
                    bt = castp.tile([P, D], bf16, name=f"bf_{nm}_{j}", tag="cast")
                    nc.vector.tensor_copy(out=bt, in_=st[:, j, :])
                    planes.append(bt)
                return planes

            def drain_copy(dst, ps_src):
                if drain_eng[0] % 2 == 0:
                    nc.vector.tensor_copy(out=dst, in_=ps_src)
                else:
                    nc.scalar.copy(out=dst, in_=ps_src)
                drain_eng[0] += 1

            def transpose_plane(plane_bf, nm, drain_to):
                """PE-transpose the 8 128x128 blocks of a [128, D] bf16 tile
                into one PSUM bank; drain_to(ps) issues the drain copy(s)."""
                ps = ps_pool.tile(
                    [P, N_DT, P], bf16, name=f"tp_{nm}", tag="pt", bufs=3
                )
                for c in range(N_DT):
                    nc.tensor.transpose(
                        ps[:, c, :], plane_bf[:, c * P : (c + 1) * P], ident_b
                    )
                drain_to(ps)

            wv_planes = {}

            def wv_dma(pk):
                wv_planes[pk] = load_pack(wv, NPK_WV, RP2, pk, f"wv{pk}")

            def wv_transp(pk, j):
                # transpose col f holds e = 256*pk + 2f + j: unscramble into
                # natural e order with strided drain copies.
                def drain(ps, pk=pk, j=j):
                    for g in range(2):
                        dst = wvg2[g].rearrange(
                            "p c (q f j) -> p c q f j", q=NPK_WV, f=P, j=RP2
                        )[:, :, pk, :, j]
                        drain_copy(dst, ps[:, 4 * g : 4 * (g + 1), :])
                transpose_plane(wv_planes[pk][j], f"wv{pk}_{j}", drain)

            ctx_planes = {}

            def ctx_dma(pk):
                ctx_planes[pk] = load_pack(ctxt, NPK_CTX, RP2, pk, f"c{pk}")

            def ctx_transp(pk):
                for j in range(RP2):
                    def drain(ps, pk=pk, j=j):
                        drain_copy(ctxT[RP2 * pk + j], ps)
                    transpose_plane(ctx_planes[pk][j], f"c{pk}_{j}", drain)

            def mm_chunk(dst, dst_sl, stat_of, mov_of, n_acc, nm, width=512):
                """One [128, <=512] output chunk accumulated over n_acc
                matmuls, drained to dst[dst_sl] (bf16) by the scalar engine."""
                ps = ps_pool.tile([P, width], fp32, name=f"ps_{nm}", tag="mm", bufs=5)
                for a in range(n_acc):
                    nc.tensor.matmul(
                        ps, stat_of(a), mov_of(a), start=(a == 0), stop=(a == n_acc - 1)
                    )
                nc.scalar.copy(out=dst[:, dst_sl], in_=ps)

            def v_ne(tb, ne):
                mm_chunk(
                    v[tb],
                    slice(ne * 512, (ne + 1) * 512),
                    lambda c: ctxT[tb][:, c, :],
                    lambda c: wvg2[c // 4][:, c % 4, ne * 512 : (ne + 1) * 512],
                    N_DT,
                    f"v{tb}_{ne}",
                )

            def v_q(tb, q):
                # quarter-width v chunk: e cols [256q, 256q+256) need only wv
                # pack q, so early t-blocks start before wv fully lands
                mm_chunk(
                    v[tb],
                    slice(q * 256, (q + 1) * 256),
                    lambda c: ctxT[tb][:, c, :],
                    lambda c: wvg2[c // 4][:, c % 4, q * 256 : (q + 1) * 256],
                    N_DT,
                    f"vq{tb}_{q}",
                    width=256,
                )

            # ---- staging schedule: hand-ordered against pack arrival times
            # (1MB every ~3.45us from ~6.6us) so the PE never queues a wait
            # on a pack that lands later than work it already has ----
            wv_dma(0)
            wv_dma(1)
            ctx_dma(0)
            ctx_dma(1)
            wv_dma(2)
            wv_dma(3)
            ctx_dma(2)
            ctx_dma(3)

            wv_transp(0, 0)
            wv_transp(0, 1)
            filler(45)
            wv_transp(1, 0)
            wv_transp(1, 1)
            filler(25)
            ctx_transp(0)
            v_q(0, 0)
            v_q(0, 1)
            v_q(1, 0)
            v_q(1, 1)
            ctx_transp(1)
            v_q(2, 0)
            v_q(2, 1)
            v_q(3, 0)
            v_q(3, 1)
            filler(32)
            wv_transp(2, 0)
            wv_transp(2, 1)
            v_q(0, 2)
            v_q(1, 2)
            filler(20)
            wv_transp(3, 0)
            wv_transp(3, 1)
            v_q(0, 3)
            v_q(1, 3)

            # ---- Wq/Wk natural-layout planes (e scrambled consistently):
            # plane (pk, j) holds e = 256*pk + 2*p + j on partition p.
            def load_w_pack(nm, dram_t, pk, lst):
                st = stage.tile([P, RP2, D], fp32, name=f"st_{nm}{pk}", tag="stage")
                src = dram_t.rearrange("(k p j) d -> k p j d", k=NPK_W, p=P, j=RP2)
                in_dma(st, src[pk])
                for j in range(RP2):
                    bt = wnp.tile([P, D], bf16, name=f"{nm}{pk}_{j}", tag="wn")
                    nc.vector.tensor_copy(out=bt, in_=st[:, j, :])
                    lst.append(bt)

            x_planes = {}

            def x_dma(pk):
                x_planes[pk] = load_pack(x, NPK_X, RP2, pk, f"x{pk}")

            # DMA issue order: the wq/wk packs interleave into the ctx4-7
            # stream so the W' inputs have landed by the time the PE drains
            # the v phase (~87us); x packs follow.
            wqn, wkn = [], []
            load_w_pack("wq", wq, 0, wqn)
            load_w_pack("wq", wq, 1, wqn)
            ctx_dma(4)
            load_w_pack("wq", wq, 2, wqn)
            ctx_dma(5)
            load_w_pack("wq", wq, 3, wqn)
            ctx_dma(6)
            load_w_pack("wk", wk, 0, wkn)
            ctx_dma(7)
            load_w_pack("wk", wk, 1, wkn)
            load_w_pack("wk", wk, 2, wkn)
            load_w_pack("wk", wk, 3, wkn)
            for pk in range(NPK_X):
                x_dma(pk)

            # PE: transposes go FIRST as each ctx pack lands (the castp ring
            # gates later casts on these reads), then that pack's v chunks.
            ctx_transp(2)
            v_ne(4, 0)
            v_ne(4, 1)
            v_ne(5, 0)
            v_ne(5, 1)
            ctx_transp(3)
            v_ne(6, 0)
            v_ne(6, 1)
            v_ne(7, 0)
            v_ne(7, 1)
            # tb0-3 ne1 quarters held back as reserve: they depend only on
            # the early wv packs, so they fill the arrival jitter of the
            # ctx4-7/wq/wk stream (which runs at 240-300GB/s run-to-run)
            v_q(2, 2)
            v_q(3, 2)
            ctx_transp(4)
            v_ne(8, 0)
            v_ne(8, 1)
            v_ne(9, 0)
            v_ne(9, 1)
            v_q(2, 3)
            ctx_transp(5)
            v_ne(10, 0)
            v_ne(10, 1)
            v_ne(11, 0)
            v_ne(11, 1)
            v_q(3, 3)
            ctx_transp(6)
            v_ne(12, 0)
            v_ne(12, 1)
            v_ne(13, 0)
            v_ne(13, 1)
            ctx_transp(7)
            for tb in range(14, N_TB):
                v_ne(tb, 0)
                v_ne(tb, 1)
            # insurance against a slow wk tail on the shared HBM: on slow-DMA
            # runs the wk3 wait reaches ~6us, enough to re-throttle the HAM;
            # even on fast runs ~1us of wait remains after these dummies, so
            # they displace idle time, not work.
            filler(45)

            xtb = [
                big8.tile([P, 4, N_DT, P], bf16, name=f"xtb{sb}", tag="big")
                for sb in range(4)
            ]

            def x_transp(pk):
                for j in range(RP2):
                    sb, jj = pk // 2, 2 * (pk % 2) + j
                    def drain(ps, sb=sb, jj=jj):
                        drain_copy(xtb[sb][:, jj, :, :], ps)
                    transpose_plane(x_planes[pk][j], f"x{pk}_{j}", drain)

            wpb = [wpp.tile([P, D], bf16, name=f"wp{i}", tag="wp") for i in range(N_DT)]

            # W' = Wq^T @ Wk interleaved with x^T staging for s-half 0
            # (packs 0-3); packs 4-7 transpose during dots(h0) instead, when
            # their DMAs have long since landed.
            for it in range(N_DT):
                for jn in range(NPROJ):
                    mm_chunk(
                        wpb[it],
                        slice(jn * 512, (jn + 1) * 512),
                        lambda e: wqn[e][:, it * P : (it + 1) * P],
                        lambda e: wkn[e][:, jn * 512 : (jn + 1) * 512],
                        N_DT,
                        f"wp{it}_{jn}",
                    )
                if it % 2 == 1:
                    x_transp(it // 2)

            # yt[jt] [128, 1024] holds yT d-chunk jt for one s-half; s columns
            # in scrambled order q = 512*sb + 128*jj + f <-> x row
            # 512*sb + 256*(jj//2) + 2*f + (jj%2).
            def yt_half(h):
                tiles = [
                    ytp.tile([P, SH], bf16, name=f"yt{h}_{jt}", tag="yt")
                    for jt in range(N_DT)
                ]
                for sb in (2 * h, 2 * h + 1):
                    for jt in range(N_DT):
                        mm_chunk(
                            tiles[jt],
                            slice((sb % 2) * 512, (sb % 2 + 1) * 512),
                            lambda c: wpb[c][:, jt * P : (jt + 1) * P],
                            lambda c: xtb[sb][:, :, c, :],
                            N_DT,
                            f"yt{h}_{jt}_{sb}",
                        )
                return tiles

            yt0 = yt_half(0)

            # ---------- attention, two s-halves ----------
            def dots_exp(h, yth, xwork=None):
                """dots^T via ctx^T x yT contraction; exp straight out of PSUM
                on the scalar engine with the 1/32 scale folded in. xwork maps
                tb -> callable run after that t-block's chains (x^T staging
                for the second s-half rides inside dots(h0))."""
                atT = []
                for tb in range(N_TB):
                    at = wnp.tile([P, SH], bf16, name=f"atT{h}_{tb}", tag="wn")
                    for ns in range(SH // 512):
                        ps = ps_pool.tile(
                            [P, 512], fp32, name=f"pd{h}_{tb}_{ns}", tag="mm", bufs=5
                        )
                        for c in range(N_DT):
                            nc.tensor.matmul(
                                ps,
                                ctxT[tb][:, c, :],
                                yth[c][:, ns * 512 : (ns + 1) * 512],
                                start=(c == 0),
                                stop=(c == N_DT - 1),
                            )
                        nc.scalar.activation(
                            out=at[:, ns * 512 : (ns + 1) * 512],
                            in_=ps,
                            func=mybir.ActivationFunctionType.Exp,
                            scale=SCALE,
                        )
                    atT.append(at)
                    if xwork and tb in xwork:
                        xwork[tb]()
                return atT

            def softmax_denoms(h, atT):
                """Column sums of attn^T: the (idle) DVE tree-reduces tile
                pairs then quads, so the PE ones-matmul chain is 4 deep; sums
                are flipped into per-partition [128, 1] reciprocals."""
                pairs = []
                for u in range(N_TB // 2):
                    s = sump.tile([P, SH], bf16, name=f"as{h}_{u}", tag="as")
                    nc.vector.tensor_tensor(
                        s, atT[2 * u], atT[2 * u + 1], mybir.AluOpType.add
                    )
                    pairs.append(s)
                # in-place reduction rounds (no SBUF headroom for separate
                # pools): pairs[2u] += pairs[2u+1], then pairs[4u] += pairs[4u+2]
                quads = []
                for u in range(N_TB // 4):
                    nc.vector.tensor_tensor(
                        pairs[2 * u], pairs[2 * u], pairs[2 * u + 1],
                        mybir.AluOpType.add,
                    )
                    quads.append(pairs[2 * u])
                octs = []
                for u in range(N_TB // 8):
                    nc.vector.tensor_tensor(
                        quads[2 * u], quads[2 * u], quads[2 * u + 1],
                        mybir.AluOpType.add,
                    )
                    octs.append(quads[2 * u])
                srows = []
                for ns in range(SH // 512):
                    pst = ps_pool.tile(
                        [P, 512], fp32, name=f"pss{h}_{ns}", tag="mm", bufs=5
                    )
                    pss = pst[0:1, :]
                    for u in range(N_TB // 8):
                        nc.tensor.matmul(
                            pss,
                            ones_b,
                            octs[u][:, ns * 512 : (ns + 1) * 512],
                            start=(u == 0),
                            stop=(u == N_TB // 8 - 1),
                        )
                    srow = smp.tile([1, 512], bf16, name=f"srow{h}_{ns}", tag="srow")
                    nc.vector.tensor_copy(out=srow, in_=pss)
                    srows.append(srow)
                recips = []
                for sl in range(8):
                    ns, off = sl // 4, (sl % 4) * P
                    pct = ps_pool.tile(
                        [P, N_DT, P], bf16, name=f"psc{h}_{sl}", tag="pt", bufs=3
                    )
                    psc = pct[:, 0, 0:1]
                    nc.tensor.transpose(psc, srows[ns][0:1, off : off + P], ident_1)
                    recip = smp.tile(
                        [P, 1], fp32, name=f"rc{h}_{sl}", tag="recip", bufs=8
                    )
                    nc.vector.reciprocal(out=recip, in_=psc)
                    recips.append(recip)
                return recips

            # out row for block sl = 4*sb + jj at partition f:
            #   512*sb + 256*(jj//2) + 2*f + (jj%2)
            out_r = out.rearrange("(sb a f b) d -> sb a f b d", sb=4, a=2, f=P, b=RP2)

            def attnv(h, atT, recips):
                for sbh in range(2):
                    sb = 2 * h + sbh
                    for jj in range(4):
                        sl = 4 * sbh + jj
                        # alternate DMA queues so back-to-back 1MB drains
                        # don't serialize; the last s-block ships per-ne
                        # 256KB chunks to shave the exit tail.
                        fine = h == 1 and sbh == 1
                        # fine chunks ship from the scalar queue: the DMA
                        # trigger follows the mul on the same engine, skipping
                        # the cross-engine semaphore hop in the exit tail
                        eng = (
                            nc.scalar if fine
                            else (nc.sync if jj % 2 == 0 else nc.gpsimd)
                        )
                        o = stage.tile([P, D], fp32, name=f"o{h}_{sl}", tag="stage")
                        width = 256 if (fine and jj == 3) else 512
                        for nq in range(D // width):
                            ps = ps_pool.tile(
                                [P, width], fp32, name=f"pav{h}_{sl}_{nq}", tag="mm",
                                bufs=5,
                            )
                            for tb in range(N_TB):
                                nc.tensor.matmul(
                                    ps,
                                    atT[tb][:, sl * P : (sl + 1) * P],
                                    v[tb][:, nq * width : (nq + 1) * width],
                                    start=(tb == 0),
                                    stop=(tb == N_TB - 1),
                                )
                            nc.scalar.mul(
                                out=o[:, nq * width : (nq + 1) * width],
                                in_=ps,
                                mul=recips[sl],
                            )
                            if fine:
                                eng.dma_start(
                                    out=out_r[
                                        sb, jj // 2, :, jj % 2,
                                        nq * width : (nq + 1) * width,
                                    ],
                                    in_=o[:, nq * width : (nq + 1) * width],
                                )
                        if not fine:
                            eng.dma_start(
                                out=out_r[sb, jj // 2, :, jj % 2, :], in_=o
                            )

            atT0 = dots_exp(
                0,
                yt0,
                {4 * k + 3: (lambda k=k: x_transp(4 + k)) for k in range(4)},
            )
            yt1 = yt_half(1)  # fills the PE while the scalar engine exps h0
            rec0 = softmax_denoms(0, atT0)
            attnv(0, atT0, rec0)
            atT1 = dots_exp(1, yt1)
            rec1 = softmax_denoms(1, atT1)
            attnv(1, atT1, rec1)


def build_nc():
    import concourse.mybir as mybir
    import concourse.tile as tile
    from concourse import bacc

    fp32 = mybir.dt.float32
    nc = bacc.Bacc("TRN2", target_bir_lowering=False, debug=False)
    x = nc.dram_tensor("x", [S, D], fp32, kind="ExternalInput").ap()
    ctxt = nc.dram_tensor("context", [T, D], fp32, kind="ExternalInput").ap()
    wq = nc.dram_tensor("Wq", [D, D], fp32, kind="ExternalInput").ap()
    wk = nc.dram_tensor("Wk", [D, D], fp32, kind="ExternalInput").ap()
    wv = nc.dram_tensor("Wv", [D, D], fp32, kind="ExternalInput").ap()
    out = nc.dram_tensor("out", [S, D], fp32, kind="ExternalOutput").ap()
    with tile.TileContext(nc) as tc:
        _emit_body(tc, x, ctxt, wq, wk, wv, out)
    nc.compile()
    return nc


def make_in_maps(inputs):
    x = np.ascontiguousarray(np.asarray(inputs["x"], dtype=np.float32))
    ctxt = np.ascontiguousarray(np.asarray(inputs["context"], dtype=np.float32))
    wq = np.ascontiguousarray(np.asarray(inputs["Wq"], dtype=np.float32))
    wk = np.ascontiguousarray(np.asarray(inputs["Wk"], dtype=np.float32))
    wv = np.ascontiguousarray(np.asarray(inputs["Wv"], dtype=np.float32))
    return [
        {"x": x[b], "context": ctxt[b], "Wq": wq, "Wk": wk, "Wv": wv}
        for b in range(B)
    ]


_CACHED_NC = None


def kernel(**inputs):
    global _CACHED_NC
    from concourse.bass_utils import run_bass_kernel_spmd

    if _CACHED_NC is None:
        _CACHED_NC = build_nc()
    nc = _CACHED_NC

    res = run_bass_kernel_spmd(nc, make_in_maps(inputs), core_ids=list(range(B)))
    return np.stack([res.results[b]["out"] for b in range(B)], axis=0)
